# revision 26
# baseline (speedup 1.0000x reference)
"""Causal self-attention on 8 Trainium2 NeuronCores.

Problem: B=2, T=2048, E=1024, H=16 heads (D=64), fp32.
  qkv = x @ W_qkv + b_qkv ; causal softmax attention ; y @ W_out + b_out

Sharding (per the hint): core c handles batch b = c//4 and head group
g = c%4 (4 heads, 256 of the 1024 hidden dims).  QKV + attention are
computed fully locally per core (tensor-parallel on heads, data-parallel
on batch).  The pre-projection outputs y_local.T [256, 512] per q-tile
are AllGather-ed within each batch group of 4 cores (pipelined, one
collective per 512-wide q-tile so communication overlaps attention of
the next tile), after which every core applies W_out[:, own 256 cols]
to the full y (Megatron-style column split of the output projection).
Host-side work is only slicing / transposition for layout and the final
concatenation.

Matmul operands are cast on-device to DT (bf16 by default: fast weight
loads + full-rate PE; float32r available for higher precision).  PSUM
accumulation is fp32 throughout.  Attention uses the transposed-scores
layout: S.T[k, q] tiles so the softmax denominator comes from an
appended ones-column in the V stationary operand and exp() runs on the
Scalar engine straight out of PSUM.  Causal masking is an additive
-1e9 on the (at most four) diagonal k-chunks of each q-tile; the
softmax reciprocal is computed as exp(-ln(sum)) on the Scalar engine.
"""

import numpy as np

import concourse.bass as bass
import concourse.mybir as mybir
import concourse.tile as tile
from concourse import bacc
from concourse.bass_utils import run_bass_kernel_spmd

F32 = mybir.dt.float32
F32R = mybir.dt.float32r
BF16 = mybir.dt.bfloat16
AF = mybir.ActivationFunctionType
OP = mybir.AluOpType

B, T, E, H = 2, 2048, 1024, 16
D = E // H            # 64 head dim
HL = 4                # heads per core
DL = HL * D           # 256 local hidden dims per core
NE = E // 128         # 8 contraction chunks
NT = T // 128         # 16 t-chunks
NJ = T // 512         # 4 q-tiles
SCALE = 1.0 / float(np.sqrt(D))
NEG = -1.0e9

_CACHE = {}


def build_bass(dbg=False, dt=BF16):
    nc = bacc.Bacc("TRN2", target_bir_lowering=False, debug=False, num_devices=8)

    # For the float32r variant the fp32 inputs are loaded directly into
    # f32r-typed tiles (bit-identical); bf16 loads fp32 then casts on DVE.
    indt = F32R if dt == F32R else F32
    xT = nc.dram_tensor("xT", [E, T], indt, kind="ExternalInput")
    wqkv = nc.dram_tensor("wqkv", [E, 3 * DL], indt, kind="ExternalInput")
    bqkv = nc.dram_tensor("bqkv", [6, 128], F32, kind="ExternalInput")
    wout = nc.dram_tensor("wout", [E, DL], indt, kind="ExternalInput")
    bout = nc.dram_tensor("bout", [2, 128], F32, kind="ExternalInput")
    ones2 = nc.dram_tensor("ones2", [128, NT * HL], dt, kind="ExternalInput")
    out_shard = nc.dram_tensor("out_shard", [DL, T], F32, kind="ExternalOutput")

    # per-q-tile staging for the pipelined AllGather (j-major, contiguous)
    ylocal = nc.dram_tensor("ylocal", [NJ, DL, 512], dt)
    ytfull = nc.dram_tensor("ytfull", [NJ, E, 512], dt)
    if dbg:
        ddt = F32 if dt == F32R else dt
        dbg_qt = nc.dram_tensor("dbg_qt", [128, 2, T], ddt, kind="ExternalOutput")
        dbg_kt = nc.dram_tensor("dbg_kt", [128, 2, T], ddt, kind="ExternalOutput")
        dbg_v = nc.dram_tensor("dbg_v", [128, NT, HL, D + 1], ddt, kind="ExternalOutput")
        dbg_yl = nc.dram_tensor("dbg_yl", [NJ, DL, 512], ddt, kind="ExternalOutput")

    with tile.TileContext(nc) as tc:
        with tc.tile_pool(name="const", bufs=1) as constp:
            bq_s = constp.tile([128, 6], F32)
            nc.gpsimd.dma_start(out=bq_s[:], in_=bqkv.ap().rearrange("m p -> p m"))
            bo_s = constp.tile([128, 2], F32)
            nc.gpsimd.dma_start(out=bo_s[:], in_=bout.ap().rearrange("m p -> p m"))
            # single [128, 128] additive triangle for the diagonal blocks
            tri = constp.tile([128, 128], F32)
            nc.gpsimd.memset(tri[:], 0.0)
            nc.gpsimd.affine_select(
                out=tri[:], in_=tri[:], compare_op=OP.is_ge, fill=NEG,
                base=0, pattern=[[1, 128]], channel_multiplier=-1)
            ones_s = constp.tile([1, 64], dt)

            with tc.tile_pool(name="qkvs", bufs=1) as qkvp:
                QT_s = qkvp.tile([128, 2, T], dt)
                KT_s = qkvp.tile([128, 2, T], dt)
                V_s = qkvp.tile([128, NT, HL, D + 1], dt)
                wo_s = qkvp.tile([128, NE, DL], dt)
                nc.gpsimd.dma_start(
                    out=V_s[:, :, :, D],
                    in_=ones2.ap().rearrange("p (a b) -> p a b", a=NT))
                nc.gpsimd.dma_start(out=ones_s[:], in_=ones2[0:1, 0:64])

                # ---------------- phase 1: QKV projections ----------------
                with tc.tile_pool(name="in1", bufs=1) as in1, \
                     tc.tile_pool(name="ps1", bufs=4, space="PSUM") as ps1, \
                     tc.tile_pool(name="ps1v", bufs=2, space="PSUM") as ps1v:
                    xT_r = xT.ap().rearrange("(c p) t -> p c t", p=128)
                    wq_r = wqkv.ap().rearrange("(c p) m -> p c m", p=128)
                    wo_r = wout.ap().rearrange("(c p) m -> p c m", p=128)
                    if dt == F32R:
                        x_s = in1.tile([128, NE, T], F32R, tag="xb")
                        nc.sync.dma_start(out=x_s[:], in_=xT_r)
                        w_s = in1.tile([128, NE, 3 * DL], F32R, tag="wb")
                        nc.sync.dma_start(out=w_s[:], in_=wq_r)
                        nc.sync.dma_start(out=wo_s[:], in_=wo_r)
                    else:
                        # per-chunk DMA + cast so QKV matmuls start early
                        xf = in1.tile([128, NE, T], F32, tag="xf")
                        x_s = in1.tile([128, NE, T], dt, tag="xb")
                        wf = in1.tile([128, NE, 3 * DL], F32, tag="wf")
                        w_s = in1.tile([128, NE, 3 * DL], dt, tag="wb")
                        wof = in1.tile([128, NE, DL], F32, tag="wof")
                        for ec in range(NE):
                            nc.sync.dma_start(out=wf[:, ec, :], in_=wq_r[:, ec, :])
                            nc.vector.tensor_copy(w_s[:, ec, :], wf[:, ec, :])
                            nc.sync.dma_start(out=xf[:, ec, :], in_=xT_r[:, ec, :])
                            nc.vector.tensor_copy(x_s[:, ec, :], xf[:, ec, :])
                        for ec in range(NE):
                            nc.sync.dma_start(out=wof[:, ec, :], in_=wo_r[:, ec, :])
                            nc.vector.tensor_copy(wo_s[:, ec, :], wof[:, ec, :])

                    # Q.T / K.T : [cols 256, T] each, cols on partitions
                    for m in range(4):
                        dest = QT_s if m < 2 else KT_s
                        mm = m % 2
                        pss = [ps1.tile([128, 512], F32, tag="psQ", name=f"psQ{m}_{_i}")
                               for _i in range(NJ)]
                        for ec in range(NE):
                            for nt in range(NJ):
                                nc.tensor.matmul(
                                    pss[nt][:],
                                    w_s[:, ec, m * 128:(m + 1) * 128],
                                    x_s[:, ec, nt * 512:(nt + 1) * 512],
                                    start=(ec == 0), stop=(ec == NE - 1))
                        for nt in range(NJ):
                            nc.scalar.activation(
                                dest[:, mm, nt * 512:(nt + 1) * 512], pss[nt][:],
                                AF.Identity, bias=bq_s[:, m:m + 1], scale=1.0)
                    # V natural [t, vcol] (bias folded in later via row-sums)
                    for mt in range(NT):
                        ps = ps1v.tile([128, 256], F32, tag="psV")
                        for ec in range(NE):
                            nc.tensor.matmul(
                                ps[:],
                                x_s[:, ec, mt * 128:(mt + 1) * 128],
                                w_s[:, ec, 2 * DL:3 * DL],
                                start=(ec == 0), stop=(ec == NE - 1))
                        nc.scalar.activation(
                            V_s[:, mt, :, 0:D],
                            ps[:].rearrange("p (a b) -> p a b", a=HL),
                            AF.Identity, bias=0.0, scale=1.0)

                # ------- phase 2+3: attention, pipelined AllGather, out proj -------
                with tc.tile_pool(name="attn", bufs=2) as attnp, \
                     tc.tile_pool(name="out3", bufs=3) as out3, \
                     tc.tile_pool(name="psS", bufs=2, space="PSUM") as psSp, \
                     tc.tile_pool(name="psO", bufs=1, space="PSUM") as psOp, \
                     tc.tile_pool(name="psR", bufs=2, space="PSUM") as psRp, \
                     tc.tile_pool(name="ps3", bufs=1, space="PSUM") as ps3:
                    for j in range(NJ):
                        OTn = attnp.tile([128, 2, 512], dt, tag="otn")
                        nkc = 4 * j + 4
                        poS = [attnp.tile([D + 1, 512], F32, tag="poS",
                                          name=f"poS{j}_{_h}") for _h in range(HL)]
                        for h in range(HL):
                            b64 = 64 * (h % 2)
                            hh = h // 2
                            expS = attnp.tile([128, NT, 512], dt, tag="expS")
                            for kp in range(nkc // 2):
                                ps = psSp.tile([128, 2, 512], F32, tag="psS")
                                for half in range(2):
                                    kc = 2 * kp + half
                                    # columns q' < off are fully masked: skip
                                    # them in the matmul, memset -1e9 instead
                                    off = max(0, 128 * kc - 512 * j)
                                    nc.tensor.matmul(
                                        ps[:, half, off:512],
                                        KT_s[b64:b64 + 64, hh, kc * 128:(kc + 1) * 128],
                                        QT_s[b64:b64 + 64, hh,
                                             j * 512 + off:(j + 1) * 512],
                                        start=True, stop=True)
                                    if off > 0:
                                        nc.vector.memset(ps[:, half, 0:off], NEG)
                                    if off < 512 and 128 * kc >= 512 * j:
                                        # diagonal block: additive triangle
                                        nc.vector.tensor_tensor(
                                            out=ps[:, half, off:off + 128],
                                            in0=ps[:, half, off:off + 128],
                                            in1=tri[:], op=OP.add)
                                nc.scalar.activation(
                                    expS[:, 2 * kp:2 * kp + 2, :], ps[:],
                                    AF.Exp, scale=SCALE)
                            po = psOp.tile([D + 1, 512], F32, tag="psO")
                            for kc in range(nkc):
                                off = max(0, 128 * kc - 512 * j)
                                nc.tensor.matmul(
                                    po[:, off:512], V_s[:, kc, h, :],
                                    expS[:, kc, off:512],
                                    start=(kc == 0), stop=(kc == nkc - 1))
                            nc.vector.tensor_copy(poS[h][:], po[:])
                        # softmax denominators: 1/sum on DVE, broadcast across
                        # partitions with a rank-1 PE matmul (ones ⊗ r) so the
                        # GpSimd engine stays free for the collectives
                        rrows = [attnp.tile([1, 512], F32, tag="rrow",
                                            name=f"rrow{j}_{_h}") for _h in range(HL)]
                        rrbs = [attnp.tile([1, 512], dt, tag="rrb",
                                           name=f"rrb{j}_{_h}") for _h in range(HL)]
                        for h in range(HL):
                            nc.vector.reciprocal(rrows[h][:], poS[h][D:D + 1, :])
                            nc.vector.tensor_copy(rrbs[h][:], rrows[h][:])
                        for h in range(HL):
                            b64 = 64 * (h % 2)
                            hh = h // 2
                            rb = psRp.tile([64, 512], F32, tag="rb")
                            nc.tensor.matmul(rb[:], ones_s[:], rrbs[h][:],
                                             start=True, stop=True)
                            dst = OTn[b64:b64 + 64, hh, :]
                            nc.vector.tensor_tensor(
                                out=dst, in0=poS[h][0:D, :], in1=rb[:], op=OP.mult)
                            # + b_qkv V-slice (attention rows sum to 1)
                            nc.vector.tensor_scalar_add(
                                dst, dst, bq_s[b64:b64 + 64, 4 + hh:5 + hh])
                        for c2 in range(2):
                            nc.sync.dma_start(
                                out=ylocal[j, 128 * c2:128 * (c2 + 1), :],
                                in_=OTn[:, c2, :])
                            if dbg:
                                nc.sync.dma_start(
                                    out=dbg_yl[j, 128 * c2:128 * (c2 + 1), :],
                                    in_=OTn[:, c2, :].bitcast(F32) if dt == F32R
                                    else OTn[:, c2, :])
                        # all-gather this q-tile's y.T within the batch group
                        nc.gpsimd.collective_compute(
                            "AllGather", OP.bypass,
                            replica_groups=[[0, 1, 2, 3], [4, 5, 6, 7]],
                            ins=[ylocal[j]], outs=[ytfull[j]])
                        # output projection for this q-tile
                        ytj = out3.tile([128, NE, 512], dt, tag="ytj")
                        nc.sync.dma_start(
                            out=ytj[:],
                            in_=ytfull[j].rearrange("(c p) t -> p c t", p=128))
                        for mc in range(2):
                            ps = ps3.tile([128, 512], F32, tag="psP")
                            for ec in range(NE):
                                nc.tensor.matmul(
                                    ps[:],
                                    wo_s[:, ec, mc * 128:(mc + 1) * 128],
                                    ytj[:, ec, :],
                                    start=(ec == 0), stop=(ec == NE - 1))
                            ot = out3.tile([128, 512], F32, tag="ot")
                            nc.vector.tensor_scalar_add(
                                ot[:], ps[:], bo_s[:, mc:mc + 1])
                            nc.sync.dma_start(
                                out=out_shard[mc * 128:(mc + 1) * 128,
                                              j * 512:(j + 1) * 512],
                                in_=ot[:])
                    if dbg:
                        cast = (lambda ap: ap.bitcast(F32)) if dt == F32R else (lambda ap: ap)
                        nc.sync.dma_start(out=dbg_qt[:, :, :], in_=cast(QT_s[:]))
                        nc.sync.dma_start(out=dbg_kt[:, :, :], in_=cast(KT_s[:]))
                        nc.sync.dma_start(out=dbg_v[:, :, :, :], in_=cast(V_s[:]))
    nc.compile()
    return nc


def _get_nc(dbg=False, dt=BF16):
    key = ("nc", dbg, dt)
    if key not in _CACHE:
        _CACHE[key] = build_bass(dbg, dt)
    return _CACHE[key]


def kernel(x, W_qkv, b_qkv, W_out, b_out, dbg=False, dt=BF16, **run_kwargs):
    x = np.asarray(x, np.float32)
    W_qkv = np.asarray(W_qkv, np.float32)
    b_qkv = np.asarray(b_qkv, np.float32)
    W_out = np.asarray(W_out, np.float32)
    b_out = np.asarray(b_out, np.float32)

    import ml_dtypes
    ones_np_dt = ml_dtypes.bfloat16 if dt == BF16 else np.float32
    ones2 = np.ones((128, NT * HL), ones_np_dt)
    in_maps = []
    for c in range(8):
        b, g = divmod(c, 4)
        cols = slice(g * DL, (g + 1) * DL)
        wq = W_qkv[:, 0 * E:1 * E][:, cols]
        wk = W_qkv[:, 1 * E:2 * E][:, cols]
        wv = W_qkv[:, 2 * E:3 * E][:, cols]
        bq = b_qkv[0 * E:1 * E][cols]
        bk = b_qkv[1 * E:2 * E][cols]
        bv = b_qkv[2 * E:3 * E][cols]
        in_maps.append({
            "xT": np.ascontiguousarray(x[b].T),
            "wqkv": np.ascontiguousarray(np.concatenate([wq, wk, wv], axis=1)),
            "bqkv": np.concatenate([bq, bk, bv]).reshape(6, 128),
            "wout": np.ascontiguousarray(W_out[:, cols]),
            "bout": np.ascontiguousarray(b_out[cols]).reshape(2, 128),
            "ones2": ones2,
        })

    res = run_bass_kernel_spmd(_get_nc(dbg, dt), in_maps, list(range(8)), **run_kwargs)
    _CACHE["last_results"] = res

    out = np.empty((B, T, E), np.float32)
    for c in range(8):
        b, g = divmod(c, 4)
        out[b][:, g * DL:(g + 1) * DL] = res.results[c]["out_shard"].T
    return out


# revision 27
# speedup vs baseline: 1.0536x; 1.0536x over previous
"""Causal self-attention on 8 Trainium2 NeuronCores.

Problem: B=2, T=2048, E=1024, H=16 heads (D=64), fp32.
  qkv = x @ W_qkv + b_qkv ; causal softmax attention ; y @ W_out + b_out

Sharding (per the hint): core c handles batch b = c//4 and head group
g = c%4 (4 heads, 256 of the 1024 hidden dims).  QKV + attention are
computed fully locally per core (tensor-parallel on heads, data-parallel
on batch).  The pre-projection outputs y_local.T [256, 512] per q-tile
are AllGather-ed within each batch group of 4 cores (pipelined, one
collective per 512-wide q-tile so communication overlaps attention of
the next tile), after which every core applies W_out[:, own 256 cols]
to the full y (Megatron-style column split of the output projection).
Host-side work is only slicing / transposition for layout and the final
concatenation.

Matmul operands are cast on-device to DT (bf16 by default: fast weight
loads + full-rate PE; float32r available for higher precision).  PSUM
accumulation is fp32 throughout.  Attention uses the transposed-scores
layout: S.T[k, q] tiles so the softmax denominator comes from an
appended ones-column in the V stationary operand and exp() runs on the
Scalar engine straight out of PSUM.  Causal masking is an additive
-1e9 on the (at most four) diagonal k-chunks of each q-tile; the
softmax reciprocal is computed as exp(-ln(sum)) on the Scalar engine.
"""

import numpy as np

import concourse.bass as bass
import concourse.mybir as mybir
import concourse.tile as tile
from concourse import bacc
from concourse.bass_utils import run_bass_kernel_spmd

F32 = mybir.dt.float32
F32R = mybir.dt.float32r
BF16 = mybir.dt.bfloat16
AF = mybir.ActivationFunctionType
OP = mybir.AluOpType

B, T, E, H = 2, 2048, 1024, 16
D = E // H            # 64 head dim
HL = 4                # heads per core
DL = HL * D           # 256 local hidden dims per core
NE = E // 128         # 8 contraction chunks
NT = T // 128         # 16 t-chunks
NJ = T // 512         # 4 q-tiles
SCALE = 1.0 / float(np.sqrt(D))
NEG = -1.0e9

_CACHE = {}


def build_bass(dbg=False, dt=BF16):
    nc = bacc.Bacc("TRN2", target_bir_lowering=False, debug=False, num_devices=8)

    # For the float32r variant the fp32 inputs are loaded directly into
    # f32r-typed tiles (bit-identical); bf16 loads fp32 then casts on DVE.
    indt = F32R if dt == F32R else F32
    xT = nc.dram_tensor("xT", [E, T], indt, kind="ExternalInput")
    wqkv = nc.dram_tensor("wqkv", [E, 3 * DL], indt, kind="ExternalInput")
    bqkv = nc.dram_tensor("bqkv", [6, 128], F32, kind="ExternalInput")
    wout = nc.dram_tensor("wout", [E, DL], indt, kind="ExternalInput")
    bout = nc.dram_tensor("bout", [2, 128], F32, kind="ExternalInput")
    ones2 = nc.dram_tensor("ones2", [128, NT * HL], dt, kind="ExternalInput")
    out_shard = nc.dram_tensor("out_shard", [DL, T], F32, kind="ExternalOutput")

    # per-q-tile staging for the pipelined AllGather (j-major, contiguous)
    ylocal = nc.dram_tensor("ylocal", [NJ, DL, 512], dt)
    ytfull = nc.dram_tensor("ytfull", [NJ, E, 512], dt)
    if dbg:
        ddt = F32 if dt == F32R else dt
        dbg_qt = nc.dram_tensor("dbg_qt", [128, 2, T], ddt, kind="ExternalOutput")
        dbg_kt = nc.dram_tensor("dbg_kt", [128, 2, T], ddt, kind="ExternalOutput")
        dbg_v = nc.dram_tensor("dbg_v", [128, NT, HL, D + 1], ddt, kind="ExternalOutput")
        dbg_yl = nc.dram_tensor("dbg_yl", [NJ, DL, 512], ddt, kind="ExternalOutput")

    with tile.TileContext(nc) as tc:
        with tc.tile_pool(name="const", bufs=1) as constp:
            bq_s = constp.tile([128, 6], F32)
            nc.gpsimd.dma_start(out=bq_s[:], in_=bqkv.ap().rearrange("m p -> p m"))
            bo_s = constp.tile([128, 2], F32)
            nc.gpsimd.dma_start(out=bo_s[:], in_=bout.ap().rearrange("m p -> p m"))
            # single [128, 128] additive triangle for the diagonal blocks
            tri = constp.tile([128, 128], F32)
            nc.gpsimd.memset(tri[:], 0.0)
            nc.gpsimd.affine_select(
                out=tri[:], in_=tri[:], compare_op=OP.is_ge, fill=NEG,
                base=0, pattern=[[1, 128]], channel_multiplier=-1)
            ones_s = constp.tile([1, 64], dt)

            with tc.tile_pool(name="qkvs", bufs=1) as qkvp:
                QT_s = qkvp.tile([128, 2, T], dt)
                KT_s = qkvp.tile([128, 2, T], dt)
                V_s = qkvp.tile([128, NT, HL, D + 1], dt)
                wo_s = qkvp.tile([128, NE, DL], dt)
                nc.gpsimd.dma_start(
                    out=V_s[:, :, :, D],
                    in_=ones2.ap().rearrange("p (a b) -> p a b", a=NT))
                nc.gpsimd.dma_start(out=ones_s[:], in_=ones2[0:1, 0:64])

                # ---------------- phase 1: QKV projections ----------------
                with tc.tile_pool(name="in1", bufs=1) as in1, \
                     tc.tile_pool(name="ps1", bufs=4, space="PSUM") as ps1, \
                     tc.tile_pool(name="ps1v", bufs=2, space="PSUM") as ps1v:
                    xT_r = xT.ap().rearrange("(c p) t -> p c t", p=128)
                    wq_r = wqkv.ap().rearrange("(c p) m -> p c m", p=128)
                    wo_r = wout.ap().rearrange("(c p) m -> p c m", p=128)
                    if dt == F32R:
                        x_s = in1.tile([128, NE, T], F32R, tag="xb")
                        nc.sync.dma_start(out=x_s[:], in_=xT_r)
                        w_s = in1.tile([128, NE, 3 * DL], F32R, tag="wb")
                        nc.sync.dma_start(out=w_s[:], in_=wq_r)
                        nc.sync.dma_start(out=wo_s[:], in_=wo_r)
                    else:
                        # per-chunk DMA + cast so QKV matmuls start early
                        xf = in1.tile([128, NE, T], F32, tag="xf")
                        x_s = in1.tile([128, NE, T], dt, tag="xb")
                        wf = in1.tile([128, NE, 3 * DL], F32, tag="wf")
                        w_s = in1.tile([128, NE, 3 * DL], dt, tag="wb")
                        wof = in1.tile([128, NE, DL], F32, tag="wof")
                        for ec in range(NE):
                            nc.sync.dma_start(out=wf[:, ec, :], in_=wq_r[:, ec, :])
                            nc.vector.tensor_copy(w_s[:, ec, :], wf[:, ec, :])
                            nc.sync.dma_start(out=xf[:, ec, :], in_=xT_r[:, ec, :])
                            nc.vector.tensor_copy(x_s[:, ec, :], xf[:, ec, :])
                        for ec in range(NE):
                            nc.sync.dma_start(out=wof[:, ec, :], in_=wo_r[:, ec, :])
                            nc.vector.tensor_copy(wo_s[:, ec, :], wof[:, ec, :])

                    # Q.T / K.T : [cols 256, T] each, cols on partitions
                    for m in range(4):
                        dest = QT_s if m < 2 else KT_s
                        mm = m % 2
                        pss = [ps1.tile([128, 512], F32, tag="psQ", name=f"psQ{m}_{_i}")
                               for _i in range(NJ)]
                        for ec in range(NE):
                            for nt in range(NJ):
                                nc.tensor.matmul(
                                    pss[nt][:],
                                    w_s[:, ec, m * 128:(m + 1) * 128],
                                    x_s[:, ec, nt * 512:(nt + 1) * 512],
                                    start=(ec == 0), stop=(ec == NE - 1))
                        for nt in range(NJ):
                            nc.scalar.activation(
                                dest[:, mm, nt * 512:(nt + 1) * 512], pss[nt][:],
                                AF.Identity, bias=bq_s[:, m:m + 1], scale=1.0)
                    # V natural [t, vcol] (bias folded in later via row-sums)
                    for mt in range(NT):
                        ps = ps1v.tile([128, 256], F32, tag="psV")
                        for ec in range(NE):
                            nc.tensor.matmul(
                                ps[:],
                                x_s[:, ec, mt * 128:(mt + 1) * 128],
                                w_s[:, ec, 2 * DL:3 * DL],
                                start=(ec == 0), stop=(ec == NE - 1))
                        nc.scalar.activation(
                            V_s[:, mt, :, 0:D],
                            ps[:].rearrange("p (a b) -> p a b", a=HL),
                            AF.Identity, bias=0.0, scale=1.0)

                # ------- phase 2+3: attention, pipelined AllGather, out proj -------
                with tc.tile_pool(name="attn", bufs=2) as attnp, \
                     tc.tile_pool(name="out3", bufs=3) as out3, \
                     tc.tile_pool(name="psS", bufs=2, space="PSUM") as psSp, \
                     tc.tile_pool(name="psO", bufs=1, space="PSUM") as psOp, \
                     tc.tile_pool(name="psR", bufs=2, space="PSUM") as psRp, \
                     tc.tile_pool(name="ps3", bufs=1, space="PSUM") as ps3:
                    def emit_outproj(jj):
                        ytj = out3.tile([128, NE, 512], dt, tag="ytj",
                                        name=f"ytj{jj}")
                        nc.sync.dma_start(
                            out=ytj[:],
                            in_=ytfull[jj].rearrange("(c p) t -> p c t", p=128))
                        for mc in range(2):
                            ps = ps3.tile([128, 512], F32, tag="psP",
                                          name=f"psP{jj}_{mc}")
                            for ec in range(NE):
                                nc.tensor.matmul(
                                    ps[:],
                                    wo_s[:, ec, mc * 128:(mc + 1) * 128],
                                    ytj[:, ec, :],
                                    start=(ec == 0), stop=(ec == NE - 1))
                            ot = out3.tile([128, 512], F32, tag="ot",
                                           name=f"ot{jj}_{mc}")
                            nc.vector.tensor_scalar_add(
                                ot[:], ps[:], bo_s[:, mc:mc + 1])
                            nc.sync.dma_start(
                                out=out_shard[mc * 128:(mc + 1) * 128,
                                              jj * 512:(jj + 1) * 512],
                                in_=ot[:])

                    for j in range(NJ):
                        OTn = attnp.tile([128, 2, 512], dt, tag="otn")
                        nkc = 4 * j + 4
                        poS = [attnp.tile([D + 1, 512], F32, tag="poS",
                                          name=f"poS{j}_{_h}") for _h in range(HL)]
                        for h in range(HL):
                            b64 = 64 * (h % 2)
                            hh = h // 2
                            expS = attnp.tile([128, NT, 512], dt, tag="expS")
                            for kp in range(nkc // 2):
                                ps = psSp.tile([128, 2, 512], F32, tag="psS")
                                for half in range(2):
                                    kc = 2 * kp + half
                                    # columns q' < off are fully masked: skip
                                    # them in the matmul, memset -1e9 instead
                                    off = max(0, 128 * kc - 512 * j)
                                    nc.tensor.matmul(
                                        ps[:, half, off:512],
                                        KT_s[b64:b64 + 64, hh, kc * 128:(kc + 1) * 128],
                                        QT_s[b64:b64 + 64, hh,
                                             j * 512 + off:(j + 1) * 512],
                                        start=True, stop=True)
                                    if off > 0:
                                        nc.vector.memset(ps[:, half, 0:off], NEG)
                                    if off < 512 and 128 * kc >= 512 * j:
                                        # diagonal block: additive triangle
                                        nc.vector.tensor_tensor(
                                            out=ps[:, half, off:off + 128],
                                            in0=ps[:, half, off:off + 128],
                                            in1=tri[:], op=OP.add)
                                nc.scalar.activation(
                                    expS[:, 2 * kp:2 * kp + 2, :], ps[:],
                                    AF.Exp, scale=SCALE)
                            po = psOp.tile([D + 1, 512], F32, tag="psO")
                            for kc in range(nkc):
                                off = max(0, 128 * kc - 512 * j)
                                nc.tensor.matmul(
                                    po[:, off:512], V_s[:, kc, h, :],
                                    expS[:, kc, off:512],
                                    start=(kc == 0), stop=(kc == nkc - 1))
                            nc.vector.tensor_copy(poS[h][:], po[:])
                        # softmax denominators: 1/sum on DVE, broadcast across
                        # partitions with a rank-1 PE matmul (ones ⊗ r) so the
                        # GpSimd engine stays free for the collectives
                        rrows = [attnp.tile([1, 512], F32, tag="rrow",
                                            name=f"rrow{j}_{_h}") for _h in range(HL)]
                        rrbs = [attnp.tile([1, 512], dt, tag="rrb",
                                           name=f"rrb{j}_{_h}") for _h in range(HL)]
                        for h in range(HL):
                            nc.vector.reciprocal(rrows[h][:], poS[h][D:D + 1, :])
                            nc.vector.tensor_copy(rrbs[h][:], rrows[h][:])
                        for h in range(HL):
                            b64 = 64 * (h % 2)
                            hh = h // 2
                            rb = psRp.tile([64, 512], F32, tag="rb")
                            nc.tensor.matmul(rb[:], ones_s[:], rrbs[h][:],
                                             start=True, stop=True)
                            dst = OTn[b64:b64 + 64, hh, :]
                            nc.vector.tensor_tensor(
                                out=dst, in0=poS[h][0:D, :], in1=rb[:], op=OP.mult)
                            # + b_qkv V-slice (attention rows sum to 1)
                            nc.vector.tensor_scalar_add(
                                dst, dst, bq_s[b64:b64 + 64, 4 + hh:5 + hh])
                        for c2 in range(2):
                            nc.sync.dma_start(
                                out=ylocal[j, 128 * c2:128 * (c2 + 1), :],
                                in_=OTn[:, c2, :])
                            if dbg:
                                nc.sync.dma_start(
                                    out=dbg_yl[j, 128 * c2:128 * (c2 + 1), :],
                                    in_=OTn[:, c2, :].bitcast(F32) if dt == F32R
                                    else OTn[:, c2, :])
                        # all-gather this q-tile's y.T within the batch group
                        nc.gpsimd.collective_compute(
                            "AllGather", OP.bypass,
                            replica_groups=[[0, 1, 2, 3], [4, 5, 6, 7]],
                            ins=[ylocal[j]], outs=[ytfull[j]])
                        # output projection for the PREVIOUS q-tile: emitted
                        # here so its matmuls sit after this tile's attention
                        # in the PE stream — the collective latency is hidden
                        if j > 0:
                            emit_outproj(j - 1)
                    emit_outproj(NJ - 1)
                    if dbg:
                        cast = (lambda ap: ap.bitcast(F32)) if dt == F32R else (lambda ap: ap)
                        nc.sync.dma_start(out=dbg_qt[:, :, :], in_=cast(QT_s[:]))
                        nc.sync.dma_start(out=dbg_kt[:, :, :], in_=cast(KT_s[:]))
                        nc.sync.dma_start(out=dbg_v[:, :, :, :], in_=cast(V_s[:]))
    nc.compile()
    return nc


def _get_nc(dbg=False, dt=BF16):
    key = ("nc", dbg, dt)
    if key not in _CACHE:
        _CACHE[key] = build_bass(dbg, dt)
    return _CACHE[key]


def kernel(x, W_qkv, b_qkv, W_out, b_out, dbg=False, dt=BF16, **run_kwargs):
    x = np.asarray(x, np.float32)
    W_qkv = np.asarray(W_qkv, np.float32)
    b_qkv = np.asarray(b_qkv, np.float32)
    W_out = np.asarray(W_out, np.float32)
    b_out = np.asarray(b_out, np.float32)

    import ml_dtypes
    ones_np_dt = ml_dtypes.bfloat16 if dt == BF16 else np.float32
    ones2 = np.ones((128, NT * HL), ones_np_dt)
    in_maps = []
    for c in range(8):
        b, g = divmod(c, 4)
        cols = slice(g * DL, (g + 1) * DL)
        wq = W_qkv[:, 0 * E:1 * E][:, cols]
        wk = W_qkv[:, 1 * E:2 * E][:, cols]
        wv = W_qkv[:, 2 * E:3 * E][:, cols]
        bq = b_qkv[0 * E:1 * E][cols]
        bk = b_qkv[1 * E:2 * E][cols]
        bv = b_qkv[2 * E:3 * E][cols]
        in_maps.append({
            "xT": np.ascontiguousarray(x[b].T),
            "wqkv": np.ascontiguousarray(np.concatenate([wq, wk, wv], axis=1)),
            "bqkv": np.concatenate([bq, bk, bv]).reshape(6, 128),
            "wout": np.ascontiguousarray(W_out[:, cols]),
            "bout": np.ascontiguousarray(b_out[cols]).reshape(2, 128),
            "ones2": ones2,
        })

    res = run_bass_kernel_spmd(_get_nc(dbg, dt), in_maps, list(range(8)), **run_kwargs)
    _CACHE["last_results"] = res

    out = np.empty((B, T, E), np.float32)
    for c in range(8):
        b, g = divmod(c, 4)
        out[b][:, g * DL:(g + 1) * DL] = res.results[c]["out_shard"].T
    return out


# revision 29
# speedup vs baseline: 1.1171x; 1.0603x over previous
"""Causal self-attention on 8 Trainium2 NeuronCores.

Problem: B=2, T=2048, E=1024, H=16 heads (D=64), fp32.
  qkv = x @ W_qkv + b_qkv ; causal softmax attention ; y @ W_out + b_out

Sharding (per the hint): core c handles batch b = c//4 and head group
g = c%4 (4 heads, 256 of the 1024 hidden dims).  QKV + attention are
computed fully locally per core (tensor-parallel on heads, data-parallel
on batch).  The pre-projection outputs y_local.T [256, 512] per q-tile
are AllGather-ed within each batch group of 4 cores (pipelined, one
collective per 512-wide q-tile so communication overlaps attention of
the next tile), after which every core applies W_out[:, own 256 cols]
to the full y (Megatron-style column split of the output projection).
Host-side work is only slicing / transposition for layout and the final
concatenation.

Matmul operands are cast on-device to DT (bf16 by default: fast weight
loads + full-rate PE; float32r available for higher precision).  PSUM
accumulation is fp32 throughout.  Attention uses the transposed-scores
layout: S.T[k, q] tiles so the softmax denominator comes from an
appended ones-column in the V stationary operand and exp() runs on the
Scalar engine straight out of PSUM.  Causal masking is an additive
-1e9 on the (at most four) diagonal k-chunks of each q-tile; the
softmax reciprocal is computed as exp(-ln(sum)) on the Scalar engine.
"""

import numpy as np

import concourse.bass as bass
import concourse.mybir as mybir
import concourse.tile as tile
from concourse import bacc
from concourse.bass_utils import run_bass_kernel_spmd

F32 = mybir.dt.float32
F32R = mybir.dt.float32r
BF16 = mybir.dt.bfloat16
AF = mybir.ActivationFunctionType
OP = mybir.AluOpType

B, T, E, H = 2, 2048, 1024, 16
D = E // H            # 64 head dim
HL = 4                # heads per core
DL = HL * D           # 256 local hidden dims per core
NE = E // 128         # 8 contraction chunks
NT = T // 128         # 16 t-chunks
NJ = T // 512         # 4 q-tiles
SCALE = 1.0 / float(np.sqrt(D))
NEG = -1.0e9

_CACHE = {}


def build_bass(dbg=False, dt=BF16):
    nc = bacc.Bacc("TRN2", target_bir_lowering=False, debug=False, num_devices=8)

    # For the float32r variant the fp32 inputs are loaded directly into
    # f32r-typed tiles (bit-identical); bf16 loads fp32 then casts on DVE.
    indt = F32R if dt == F32R else F32
    xT = nc.dram_tensor("xT", [E, T], indt, kind="ExternalInput")
    wqkv = nc.dram_tensor("wqkv", [E, 3 * DL], indt, kind="ExternalInput")
    bqkv = nc.dram_tensor("bqkv", [6, 128], F32, kind="ExternalInput")
    wout = nc.dram_tensor("wout", [E, DL], indt, kind="ExternalInput")
    bout = nc.dram_tensor("bout", [2, 128], F32, kind="ExternalInput")
    ones2 = nc.dram_tensor("ones2", [128, NT * HL], dt, kind="ExternalInput")
    out_shard = nc.dram_tensor("out_shard", [DL, T], F32, kind="ExternalOutput")

    # per-q-tile staging for the pipelined AllGather (j-major, contiguous)
    ylocal = nc.dram_tensor("ylocal", [NJ, DL, 512], dt)
    ytfull = nc.dram_tensor("ytfull", [NJ, E, 512], dt)
    if dbg:
        ddt = F32 if dt == F32R else dt
        dbg_qt = nc.dram_tensor("dbg_qt", [128, 2, T], ddt, kind="ExternalOutput")
        dbg_kt = nc.dram_tensor("dbg_kt", [128, 2, T], ddt, kind="ExternalOutput")
        dbg_v = nc.dram_tensor("dbg_v", [128, NT, HL, D + 1], ddt, kind="ExternalOutput")
        dbg_yl = nc.dram_tensor("dbg_yl", [NJ, DL, 512], ddt, kind="ExternalOutput")

    with tile.TileContext(nc) as tc:
        with tc.tile_pool(name="const", bufs=1) as constp:
            bq_s = constp.tile([128, 6], F32)
            nc.gpsimd.dma_start(out=bq_s[:], in_=bqkv.ap().rearrange("m p -> p m"))
            bo_s = constp.tile([128, 2], F32)
            nc.gpsimd.dma_start(out=bo_s[:], in_=bout.ap().rearrange("m p -> p m"))
            # single [128, 128] additive triangle for the diagonal blocks
            tri = constp.tile([128, 128], F32)
            nc.gpsimd.memset(tri[:], 0.0)
            nc.gpsimd.affine_select(
                out=tri[:], in_=tri[:], compare_op=OP.is_ge, fill=NEG,
                base=0, pattern=[[1, 128]], channel_multiplier=-1)
            ones_s = constp.tile([1, 64], dt)

            with tc.tile_pool(name="qkvs", bufs=1) as qkvp:
                QT_s = qkvp.tile([128, 2, T], dt)
                KT_s = qkvp.tile([128, 2, T], dt)
                V_s = qkvp.tile([128, NT, HL, D + 1], dt)
                wo_s = qkvp.tile([128, NE, DL], dt)
                nc.gpsimd.dma_start(
                    out=V_s[:, :, :, D],
                    in_=ones2.ap().rearrange("p (a b) -> p a b", a=NT))
                nc.gpsimd.dma_start(out=ones_s[:], in_=ones2[0:1, 0:64])

                # ---------------- phase 1: QKV projections ----------------
                with tc.tile_pool(name="in1", bufs=1) as in1, \
                     tc.tile_pool(name="ps1", bufs=8, space="PSUM") as ps1:
                    xT_r = xT.ap().rearrange("(c p) t -> p c t", p=128)
                    wq_r = wqkv.ap().rearrange("(c p) m -> p c m", p=128)
                    wo_r = wout.ap().rearrange("(c p) m -> p c m", p=128)
                    if dt == F32R:
                        x_s = in1.tile([128, NE, T], F32R, tag="xb")
                        nc.sync.dma_start(out=x_s[:], in_=xT_r)
                        w_s = in1.tile([128, NE, 3 * DL], F32R, tag="wb")
                        nc.sync.dma_start(out=w_s[:], in_=wq_r)
                        nc.sync.dma_start(out=wo_s[:], in_=wo_r)
                    else:
                        # per-chunk DMA + cast so QKV matmuls start early
                        xf = in1.tile([128, NE, T], F32, tag="xf")
                        x_s = in1.tile([128, NE, T], dt, tag="xb")
                        wf = in1.tile([128, NE, 3 * DL], F32, tag="wf")
                        w_s = in1.tile([128, NE, 3 * DL], dt, tag="wb")
                        wof = in1.tile([128, NE, DL], F32, tag="wof")
                        for ec in range(NE):
                            nc.sync.dma_start(out=wf[:, ec, :], in_=wq_r[:, ec, :])
                            nc.vector.tensor_copy(w_s[:, ec, :], wf[:, ec, :])
                            nc.sync.dma_start(out=xf[:, ec, :], in_=xT_r[:, ec, :])
                            nc.vector.tensor_copy(x_s[:, ec, :], xf[:, ec, :])
                        for ec in range(NE):
                            nc.sync.dma_start(out=wof[:, ec, :], in_=wo_r[:, ec, :])
                            nc.vector.tensor_copy(wo_s[:, ec, :], wof[:, ec, :])

                    # Q.T / K.T : [cols 256, T] each, cols on partitions.
                    # Two m-chunks at a time (8 PSUM banks) so the PE has 2x
                    # work per x-chunk while the input DMA is still streaming.
                    for mp in range(2):
                        pss = [ps1.tile([128, 512], F32, tag="psQ",
                                        name=f"psQ{mp}_{_i}") for _i in range(8)]
                        for ec in range(NE):
                            for mi in range(2):
                                m = 2 * mp + mi
                                for nt in range(NJ):
                                    nc.tensor.matmul(
                                        pss[4 * mi + nt][:],
                                        w_s[:, ec, m * 128:(m + 1) * 128],
                                        x_s[:, ec, nt * 512:(nt + 1) * 512],
                                        start=(ec == 0), stop=(ec == NE - 1))
                        for mi in range(2):
                            m = 2 * mp + mi
                            dest = QT_s if m < 2 else KT_s
                            mm = m % 2
                            for nt in range(NJ):
                                nc.scalar.activation(
                                    dest[:, mm, nt * 512:(nt + 1) * 512],
                                    pss[4 * mi + nt][:],
                                    AF.Identity, bias=bq_s[:, m:m + 1], scale=1.0)
                    # V natural [t, vcol] (bias folded in later via row-sums)
                    for mt in range(NT):
                        ps = ps1.tile([128, 256], F32, tag="psQ", name=f"psV{mt}")
                        for ec in range(NE):
                            nc.tensor.matmul(
                                ps[:],
                                x_s[:, ec, mt * 128:(mt + 1) * 128],
                                w_s[:, ec, 2 * DL:3 * DL],
                                start=(ec == 0), stop=(ec == NE - 1))
                        nc.scalar.activation(
                            V_s[:, mt, :, 0:D],
                            ps[:].rearrange("p (a b) -> p a b", a=HL),
                            AF.Identity, bias=0.0, scale=1.0)

                # ------- phase 2+3: attention, pipelined AllGather, out proj -------
                with tc.tile_pool(name="attn", bufs=2) as attnp, \
                     tc.tile_pool(name="out3", bufs=3) as out3, \
                     tc.tile_pool(name="psS", bufs=3, space="PSUM") as psSp, \
                     tc.tile_pool(name="psO", bufs=1, space="PSUM") as psOp, \
                     tc.tile_pool(name="ps3", bufs=1, space="PSUM") as ps3:
                    def emit_outproj(jj):
                        ytj = out3.tile([128, NE, 512], dt, tag="ytj",
                                        name=f"ytj{jj}")
                        nc.sync.dma_start(
                            out=ytj[:],
                            in_=ytfull[jj].rearrange("(c p) t -> p c t", p=128))
                        for mc in range(2):
                            ps = ps3.tile([128, 512], F32, tag="psP",
                                          name=f"psP{jj}_{mc}")
                            for ec in range(NE):
                                nc.tensor.matmul(
                                    ps[:],
                                    wo_s[:, ec, mc * 128:(mc + 1) * 128],
                                    ytj[:, ec, :],
                                    start=(ec == 0), stop=(ec == NE - 1))
                            ot = out3.tile([128, 512], F32, tag="ot",
                                           name=f"ot{jj}_{mc}")
                            nc.vector.tensor_scalar_add(
                                ot[:], ps[:], bo_s[:, mc:mc + 1])
                            nc.sync.dma_start(
                                out=out_shard[mc * 128:(mc + 1) * 128,
                                              jj * 512:(jj + 1) * 512],
                                in_=ot[:])

                    for j in range(NJ):
                        OTn = attnp.tile([128, 2, 512], dt, tag="otn")
                        nkc = 4 * j + 4
                        poS = [attnp.tile([D + 1, 512], F32, tag="poS",
                                          name=f"poS{j}_{_h}") for _h in range(HL)]
                        for h in range(HL):
                            b64 = 64 * (h % 2)
                            hh = h // 2
                            expS = attnp.tile([128, NT, 512], dt, tag="expS")
                            for kp in range(nkc // 2):
                                ps = psSp.tile([128, 2, 512], F32, tag="psS")
                                offs = []
                                for half in range(2):
                                    kc = 2 * kp + half
                                    # columns q' < off are fully masked: the
                                    # matmul, exp and AV all skip them
                                    off = max(0, 128 * kc - 512 * j)
                                    offs.append(off)
                                    nc.tensor.matmul(
                                        ps[:, half, off:512],
                                        KT_s[b64:b64 + 64, hh, kc * 128:(kc + 1) * 128],
                                        QT_s[b64:b64 + 64, hh,
                                             j * 512 + off:(j + 1) * 512],
                                        start=True, stop=True)
                                    if 128 * kc >= 512 * j:
                                        # diagonal block: additive triangle
                                        nc.vector.tensor_tensor(
                                            out=ps[:, half, off:off + 128],
                                            in0=ps[:, half, off:off + 128],
                                            in1=tri[:], op=OP.add)
                                if offs == [0, 0]:
                                    nc.scalar.activation(
                                        expS[:, 2 * kp:2 * kp + 2, :], ps[:],
                                        AF.Exp, scale=SCALE)
                                else:
                                    for half in range(2):
                                        kc = 2 * kp + half
                                        nc.scalar.activation(
                                            expS[:, kc, offs[half]:512],
                                            ps[:, half, offs[half]:512],
                                            AF.Exp, scale=SCALE)
                            po = psOp.tile([D + 1, 512], F32, tag="psO")
                            for kc in range(nkc):
                                off = max(0, 128 * kc - 512 * j)
                                nc.tensor.matmul(
                                    po[:, off:512], V_s[:, kc, h, :],
                                    expS[:, kc, off:512],
                                    start=(kc == 0), stop=(kc == nkc - 1))
                            nc.vector.tensor_copy(poS[h][:], po[:])
                        # softmax denominators: 1/sum on DVE, broadcast across
                        # partitions with a rank-1 PE matmul (ones ⊗ r) so the
                        # GpSimd engine stays free for the collectives
                        rrows = [attnp.tile([1, 512], F32, tag="rrow",
                                            name=f"rrow{j}_{_h}") for _h in range(HL)]
                        rbs = [attnp.tile([64, 512], F32, tag="rb",
                                          name=f"rb{j}_{_h}") for _h in range(HL)]
                        for h in range(HL):
                            nc.vector.reciprocal(rrows[h][:], poS[h][D:D + 1, :])
                        for h in range(HL):
                            nc.gpsimd.partition_broadcast(rbs[h][:], rrows[h][:])
                        for h in range(HL):
                            b64 = 64 * (h % 2)
                            hh = h // 2
                            dst = OTn[b64:b64 + 64, hh, :]
                            nc.vector.tensor_tensor(
                                out=dst, in0=poS[h][0:D, :], in1=rbs[h][:], op=OP.mult)
                            # + b_qkv V-slice (attention rows sum to 1)
                            nc.vector.tensor_scalar_add(
                                dst, dst, bq_s[b64:b64 + 64, 4 + hh:5 + hh])
                        for c2 in range(2):
                            nc.sync.dma_start(
                                out=ylocal[j, 128 * c2:128 * (c2 + 1), :],
                                in_=OTn[:, c2, :])
                            if dbg:
                                nc.sync.dma_start(
                                    out=dbg_yl[j, 128 * c2:128 * (c2 + 1), :],
                                    in_=OTn[:, c2, :].bitcast(F32) if dt == F32R
                                    else OTn[:, c2, :])
                        # all-gather this q-tile's y.T within the batch group
                        nc.gpsimd.collective_compute(
                            "AllGather", OP.bypass,
                            replica_groups=[[0, 1, 2, 3], [4, 5, 6, 7]],
                            ins=[ylocal[j]], outs=[ytfull[j]])
                        # output projection for the PREVIOUS q-tile: emitted
                        # here so its matmuls sit after this tile's attention
                        # in the PE stream — the collective latency is hidden
                        if j > 0:
                            emit_outproj(j - 1)
                    emit_outproj(NJ - 1)
                    if dbg:
                        cast = (lambda ap: ap.bitcast(F32)) if dt == F32R else (lambda ap: ap)
                        nc.sync.dma_start(out=dbg_qt[:, :, :], in_=cast(QT_s[:]))
                        nc.sync.dma_start(out=dbg_kt[:, :, :], in_=cast(KT_s[:]))
                        nc.sync.dma_start(out=dbg_v[:, :, :, :], in_=cast(V_s[:]))
    nc.compile()
    return nc


def _get_nc(dbg=False, dt=BF16):
    key = ("nc", dbg, dt)
    if key not in _CACHE:
        _CACHE[key] = build_bass(dbg, dt)
    return _CACHE[key]


def kernel(x, W_qkv, b_qkv, W_out, b_out, dbg=False, dt=BF16, **run_kwargs):
    x = np.asarray(x, np.float32)
    W_qkv = np.asarray(W_qkv, np.float32)
    b_qkv = np.asarray(b_qkv, np.float32)
    W_out = np.asarray(W_out, np.float32)
    b_out = np.asarray(b_out, np.float32)

    import ml_dtypes
    ones_np_dt = ml_dtypes.bfloat16 if dt == BF16 else np.float32
    ones2 = np.ones((128, NT * HL), ones_np_dt)
    in_maps = []
    for c in range(8):
        b, g = divmod(c, 4)
        cols = slice(g * DL, (g + 1) * DL)
        wq = W_qkv[:, 0 * E:1 * E][:, cols]
        wk = W_qkv[:, 1 * E:2 * E][:, cols]
        wv = W_qkv[:, 2 * E:3 * E][:, cols]
        bq = b_qkv[0 * E:1 * E][cols]
        bk = b_qkv[1 * E:2 * E][cols]
        bv = b_qkv[2 * E:3 * E][cols]
        in_maps.append({
            "xT": np.ascontiguousarray(x[b].T),
            "wqkv": np.ascontiguousarray(np.concatenate([wq, wk, wv], axis=1)),
            "bqkv": np.concatenate([bq, bk, bv]).reshape(6, 128),
            "wout": np.ascontiguousarray(W_out[:, cols]),
            "bout": np.ascontiguousarray(b_out[cols]).reshape(2, 128),
            "ones2": ones2,
        })

    res = run_bass_kernel_spmd(_get_nc(dbg, dt), in_maps, list(range(8)), **run_kwargs)
    _CACHE["last_results"] = res

    out = np.empty((B, T, E), np.float32)
    for c in range(8):
        b, g = divmod(c, 4)
        out[b][:, g * DL:(g + 1) * DL] = res.results[c]["out_shard"].T
    return out
